# revision 1
# baseline (speedup 1.0000x reference)
"""Trainium2 Bass kernel for BiLinearSigmoidAttention.

Reference math (per batch b, with L = length[b]):
    qn = l2norm(query), cn = l2norm(context)
    raw[q,k] = qn[q] . cn[k]            (masked: k >= L -> -1e30)
    sig = sigmoid(raw)
    den[q] = max(sum_k sig[q,k], 1)
    scores[q,k] = sig[q,k] / den[q]     (rows q >= L zeroed)
    att[q,:] = sum_k scores[q,k] * context[k,:]
    out = concat([qn, att], -1)
returns (out [B,S,2D], scores [B,S,S])

Device mapping (8 NeuronCores, pure data parallel over B=32 -> 4 per core):
  - mm1 computes scoresT [k_part, q_free] so the length mask is a
    per-partition bias fused into the ACT sigmoid.
  - denominator = ones-column matmuls sharing mm2's loaded weights,
    accumulated per q-block into tiny PSUM tiles (partition-major).
  - scores output produced by PE transposes of sigT, scaled by
    w = qmask/den during PSUM->SBUF eviction.
  - matmuls run as float32r (full-rate fp32); transposes as fp32.
"""

import numpy as np

import concourse.bacc as bacc
import concourse.mybir as mybir
import concourse.tile as tile
from concourse.bass_utils import run_bass_kernel_spmd

B, S, D = 32, 1024, 512
NCORES = 8
BPC = B // NCORES          # batches per core
P = 128                    # partitions
NT = S // P                # 8 s-tiles
ND = D // P                # 4 d-chunks
NEG = np.float32(-1e30)

F32 = mybir.dt.float32
F32R = mybir.dt.float32r
AF = mybir.ActivationFunctionType
ALU = mybir.AluOpType
AX = mybir.AxisListType


def _r(ap):
    """View an fp32 AP as float32r for full-rate PE matmuls."""
    return ap.bitcast(F32R)


def build_kernel():
    nc = bacc.Bacc("TRN2", target_bir_lowering=False, debug=False)

    q_d = nc.dram_tensor("query", [BPC, S, D], F32, kind="ExternalInput")
    c_d = nc.dram_tensor("context", [BPC, S, D], F32R, kind="ExternalInput")
    # keybias[b, p, kt] = 0 if kt*P+p < L else -1e30
    kb_d = nc.dram_tensor("keybias", [BPC, P, NT], F32, kind="ExternalInput")
    # qmask[b, p, qb] = 1 if qb*P+p < L else 0
    qm_d = nc.dram_tensor("qmask", [BPC, P, NT], F32, kind="ExternalInput")
    id_d = nc.dram_tensor("identity", [P, P], F32, kind="ExternalInput")
    idr_d = nc.dram_tensor("identity_r", [P, P], F32R, kind="ExternalInput")
    on_d = nc.dram_tensor("ones", [P, 2], F32R, kind="ExternalInput")
    out_d = nc.dram_tensor("out", [BPC, S, 2 * D], F32, kind="ExternalOutput")
    sc_d = nc.dram_tensor("scores", [BPC, S, S], F32, kind="ExternalOutput")

    with tile.TileContext(nc) as tc:
        _body(tc, q_d, c_d, kb_d, qm_d, id_d, idr_d, on_d, out_d, sc_d)
    nc.compile()
    return nc


def _body(tc, q_d, c_d, kb_d, qm_d, id_d, idr_d, on_d, out_d, sc_d):
    import os

    PHASE = int(os.environ.get("KERNEL_PHASE", "4"))
    nc = tc.nc
    from contextlib import ExitStack

    ctx = ExitStack()
    with ctx:
        const = ctx.enter_context(tc.tile_pool(name="const", bufs=1))
        qpool = ctx.enter_context(tc.tile_pool(name="q", bufs=2))
        cpool = ctx.enter_context(tc.tile_pool(name="c", bufs=2))
        tpool = ctx.enter_context(tc.tile_pool(name="t", bufs=1))
        sgpool = ctx.enter_context(tc.tile_pool(name="sg", bufs=1))
        mpool = ctx.enter_context(tc.tile_pool(name="m", bufs=2))
        spool = ctx.enter_context(tc.tile_pool(name="s", bufs=3))
        opool = ctx.enter_context(tc.tile_pool(name="o", bufs=3))
        ps1 = ctx.enter_context(tc.tile_pool(name="ps1", bufs=2, space="PSUM"))
        pst = ctx.enter_context(tc.tile_pool(name="pst", bufs=2, space="PSUM"))
        ps2 = ctx.enter_context(tc.tile_pool(name="ps2", bufs=2, space="PSUM"))
        psd = ctx.enter_context(tc.tile_pool(name="psd", bufs=2, space="PSUM"))

        ident = const.tile([P, P], F32, tag="ident")
        identr = const.tile([P, P], F32R, tag="identr")
        ones = const.tile([P, 2], F32R, tag="ones")
        nc.sync.dma_start(ident[:], id_d[:])
        nc.sync.dma_start(identr[:], idr_d[:])
        nc.sync.dma_start(ones[:], on_d[:])

        for b in range(BPC):
            # ---- load ----
            qt = qpool.tile([P, NT, D], F32, tag="qt")       # qn (in-place)
            ct = cpool.tile([P, NT, D], F32R, tag="ct")       # raw context
            kb = mpool.tile([P, NT], F32, tag="kb")
            qm = mpool.tile([P, NT], F32, tag="qm")
            nc.sync.dma_start(qt[:], q_d[b].rearrange("(t p) d -> p t d", p=P))
            nc.sync.dma_start(ct[:], c_d[b].rearrange("(t p) d -> p t d", p=P))
            nc.sync.dma_start(kb[:], kb_d[b])
            nc.sync.dma_start(qm[:], qm_d[b])

            # ---- norms ----
            ssq = mpool.tile([P, 2 * NT], F32, tag="ssq")
            inv = mpool.tile([P, 2 * NT], F32, tag="inv")
            for t in range(NT):
                scr = spool.tile([P, D], F32, tag="scr")
                nc.vector.tensor_mul(scr[:], qt[:, t], qt[:, t])
                nc.vector.reduce_sum(ssq[:, t : t + 1], scr[:], axis=AX.X)
                scr2 = spool.tile([P, D], F32, tag="scr2")
                nc.scalar.activation(
                    scr2[:], ct[:, t], AF.Square,
                    accum_out=ssq[:, NT + t : NT + t + 1],
                )
            # inv = 1/sqrt(ssq)  (norms are >0 with randn inputs)
            nrm = mpool.tile([P, 2 * NT], F32, tag="nrm")
            nc.scalar.activation(nrm[:], ssq[:], AF.Sqrt)
            nc.vector.reciprocal(inv[:], nrm[:])

            # ---- qn in place, store first half of out ----
            for t in range(NT):
                nc.vector.tensor_scalar_mul(qt[:, t], qt[:, t], inv[:, t : t + 1])
            nc.sync.dma_start(
                out_d[b, :, 0:D].rearrange("(t p) d -> p t d", p=P), qt[:]
            )

            if PHASE < 2:
                continue
            # ---- transposes: qT[d, s] and cnT[d, s] ----
            qT = tpool.tile([P, ND, S], F32R, tag="qT")
            cT = tpool.tile([P, ND, S], F32R, tag="cT")
            for t in range(NT):
                pq = pst.tile([P, ND, P], F32, tag="pt")
                pc = pst.tile([P, ND, P], F32R, tag="pt")
                for dch in range(ND):
                    nc.tensor.transpose(
                        pq[:, dch], qt[:, t, dch * P : (dch + 1) * P], ident[:]
                    )
                    nc.tensor.transpose(
                        pc[:, dch], ct[:, t, dch * P : (dch + 1) * P], identr[:]
                    )
                nc.scalar.copy(qT[:, :, t * P : (t + 1) * P], pq[:])
                nc.vector.tensor_copy(cT[:, :, t * P : (t + 1) * P], pc[:])

            if PHASE < 3:
                continue
            # ---- mm1: sigT[k, q] = sigmoid(cnT.T @ qT + keybias) ----
            sg = sgpool.tile([P, NT, S], F32R, tag="sg")
            for kt in range(NT):
                for qc in range(2):
                    acc = ps1.tile([P, 512], F32, tag="acc")
                    for dch in range(ND):
                        nc.tensor.matmul(
                            acc[:],
                            cT[:, dch, kt * P : (kt + 1) * P],
                            qT[:, dch, qc * 512 : (qc + 1) * 512],
                            start=(dch == 0),
                            stop=(dch == ND - 1),
                        )
                    # context l2-normalization folds in as the per-k scale
                    nc.scalar.activation(
                        sg[:, kt, qc * 512 : (qc + 1) * 512], acc[:],
                        AF.Sigmoid, bias=kb[:, kt : kt + 1],
                        scale=inv[:, NT + kt : NT + kt + 1],
                    )

            if PHASE < 4:
                continue
            # ---- per q-block: denominator, attended, scores out ----
            for qb in range(NT):
                att = ps2.tile([P, 512], F32, tag="att")
                dn = psd.tile([P, 2], F32, tag="dn")
                for kt in range(NT):
                    sgblk = sg[:, kt, qb * P : (qb + 1) * P]
                    nc.tensor.matmul(
                        att[:], sgblk, ct[:, kt],
                        start=(kt == 0), stop=(kt == NT - 1),
                    )
                    nc.tensor.matmul(
                        dn[:], sgblk, ones[:],
                        start=(kt == 0), stop=(kt == NT - 1),
                    )
                # w = qmask / max(den, 1)
                w = mpool.tile([P, 1], F32, tag="w")
                nc.vector.tensor_scalar_max(w[:], dn[:, 0:1], 1.0)
                nc.vector.reciprocal(w[:], w[:])
                nc.vector.tensor_mul(w[:], w[:], qm[:, qb : qb + 1])

                ao = opool.tile([P, D], F32, tag="ao")
                nc.vector.tensor_scalar_mul(ao[:], att[:], w[:])
                nc.sync.dma_start(out_d[b, qb * P : (qb + 1) * P, D : 2 * D], ao[:])

                so = opool.tile([P, S], F32, tag="so")
                for kg in range(2):
                    pt = pst.tile([P, 4, P], F32R, tag="pt")
                    for j in range(4):
                        kt = kg * 4 + j
                        nc.tensor.transpose(
                            pt[:, j], sg[:, kt, qb * P : (qb + 1) * P], identr[:]
                        )
                    eng = nc.scalar if kg == 0 else nc.vector
                    if kg == 0:
                        nc.scalar.activation(
                            so[:, kg * 512 : (kg + 1) * 512], pt[:],
                            AF.Copy, scale=w[:],
                        )
                    else:
                        nc.vector.tensor_scalar_mul(
                            so[:, kg * 512 : (kg + 1) * 512], pt[:], w[:]
                        )
                nc.sync.dma_start(sc_d[b, qb * P : (qb + 1) * P, :], so[:])


_NC_CACHE = {}


def _get_nc():
    if "nc" not in _NC_CACHE:
        _NC_CACHE["nc"] = build_kernel()
    return _NC_CACHE["nc"]


def kernel(context, query, length):
    context = np.ascontiguousarray(np.asarray(context, dtype=np.float32))
    query = np.ascontiguousarray(np.asarray(query, dtype=np.float32))
    length = np.asarray(length).astype(np.int64)

    iot = np.arange(S)
    keymask = iot[None, :] < length[:, None]                      # [B, S]
    kbH = np.where(keymask, np.float32(0.0), NEG).astype(np.float32)
    kbH = np.ascontiguousarray(kbH.reshape(B, NT, P).transpose(0, 2, 1))
    qmH = keymask.astype(np.float32)
    qmH = np.ascontiguousarray(qmH.reshape(B, NT, P).transpose(0, 2, 1))
    ident = np.eye(P, dtype=np.float32)

    in_maps = []
    for c in range(NCORES):
        sl = slice(c * BPC, (c + 1) * BPC)
        in_maps.append(
            {
                "query": np.ascontiguousarray(query[sl]),
                "context": np.ascontiguousarray(context[sl]),
                "keybias": np.ascontiguousarray(kbH[sl]),
                "qmask": np.ascontiguousarray(qmH[sl]),
                "identity": ident,
                "identity_r": ident,
                "ones": np.ones((P, 2), dtype=np.float32),
            }
        )

    nc = _get_nc()
    res = run_bass_kernel_spmd(nc, in_maps, list(range(NCORES)))
    _NC_CACHE["last_result"] = res
    out = np.concatenate([res.results[c]["out"] for c in range(NCORES)], axis=0)
    scores = np.concatenate(
        [res.results[c]["scores"] for c in range(NCORES)], axis=0
    )
    return out, scores



# revision 7
# speedup vs baseline: 1.2098x; 1.2098x over previous
"""Trainium2 Bass kernel for BiLinearSigmoidAttention.

Reference math (per batch b, with L = length[b]):
    qn = l2norm(query), cn = l2norm(context)
    raw[q,k] = qn[q] . cn[k]            (masked: k >= L -> -1e30)
    sig = sigmoid(raw)
    den[q] = max(sum_k sig[q,k], 1)
    scores[q,k] = sig[q,k] / den[q]     (rows q >= L zeroed)
    att[q,:] = sum_k scores[q,k] * context[k,:]
    out = concat([qn, att], -1)
returns (out [B,S,2D], scores [B,S,S])

Device mapping (8 NeuronCores, pure data parallel over B=32 -> 4 per core).

Engine plan per batch (PE kept dense, everything PE-side in bf16 since
walrus forbids mixing 32-bit with 16-bit matmul operands):
  - per s-tile pipeline: load -> norms -> qn (in-place fp32, stored) ->
    GpSimd casts to bf16 (qnb / cb) -> PE transposes (bf16, 1 cyc/row)
    -> evict to bf16 qT/cT, so PE work starts a few us into the kernel.
  - mm1: sigT[k,q] = sigmoid(cTb.T @ qTb + keybias), bf16 x bf16,
    weights reused across the two q-halves (kt->dch->qc loop order);
    context l2-norm folded into the sigmoid per-partition scale; sigmoid
    evicts to a bf16 sg tile (halves ldweights cost for mm2/transposes).
  - mm2: att[q,d] = sgblk.T @ cb, all bf16.  Denominator rides the same
    weights via a tiny ones matmul.
  - scores out: PE transposes of bf16 sg blocks, scaled by w=qmask/den
    during PSUM->SBUF eviction (split across ACT and DVE).
"""

import numpy as np
import ml_dtypes

import concourse.bacc as bacc
import concourse.mybir as mybir
import concourse.tile as tile
from concourse.bass_utils import run_bass_kernel_spmd

B, S, D = 32, 1024, 512
NCORES = 8
BPC = B // NCORES          # batches per core
P = 128                    # partitions
NT = S // P                # 8 s-tiles
ND = D // P                # 4 d-chunks
NEG = np.float32(-1e30)

F32 = mybir.dt.float32
F32R = mybir.dt.float32r
BF16 = mybir.dt.bfloat16
AF = mybir.ActivationFunctionType
ALU = mybir.AluOpType
AX = mybir.AxisListType


def build_kernel():
    nc = bacc.Bacc("TRN2", target_bir_lowering=False, debug=False)

    q_d = nc.dram_tensor("query", [BPC, S, D], F32, kind="ExternalInput")
    c_d = nc.dram_tensor("context", [BPC, S, D], F32, kind="ExternalInput")
    # keybias[b, p, kt] = 0 if kt*P+p < L else -1e30
    kb_d = nc.dram_tensor("keybias", [BPC, P, NT], F32, kind="ExternalInput")
    # qmask[b, p, qb] = 1 if qb*P+p < L else 0
    qm_d = nc.dram_tensor("qmask", [BPC, P, NT], F32, kind="ExternalInput")
    id_d = nc.dram_tensor("identb", [P, P], BF16, kind="ExternalInput")
    on_d = nc.dram_tensor("onesb", [P, 2], BF16, kind="ExternalInput")
    out_d = nc.dram_tensor("out", [BPC, S, 2 * D], F32, kind="ExternalOutput")
    sc_d = nc.dram_tensor("scores", [BPC, S, S], F32, kind="ExternalOutput")

    with tile.TileContext(nc) as tc:
        _body(tc, q_d, c_d, kb_d, qm_d, id_d, on_d, out_d, sc_d)
    nc.compile()
    return nc


def _body(tc, q_d, c_d, kb_d, qm_d, id_d, on_d, out_d, sc_d):
    import os

    PHASE = int(os.environ.get("KERNEL_PHASE", "4"))
    nc = tc.nc
    from contextlib import ExitStack

    ctx = ExitStack()
    with ctx:
        const = ctx.enter_context(tc.tile_pool(name="const", bufs=1))
        qpool = ctx.enter_context(tc.tile_pool(name="q", bufs=2))
        cpool = ctx.enter_context(tc.tile_pool(name="c", bufs=2))
        cbpool = ctx.enter_context(tc.tile_pool(name="cb", bufs=2))
        qbpool = ctx.enter_context(tc.tile_pool(name="qb", bufs=3))
        tpool = ctx.enter_context(tc.tile_pool(name="t", bufs=2))
        sgpool = ctx.enter_context(tc.tile_pool(name="sg", bufs=2))
        mpool = ctx.enter_context(tc.tile_pool(name="m", bufs=2))
        spool = ctx.enter_context(tc.tile_pool(name="s", bufs=2))
        opool = ctx.enter_context(tc.tile_pool(name="o", bufs=3))
        wpool = ctx.enter_context(tc.tile_pool(name="w", bufs=4))
        ps1 = ctx.enter_context(tc.tile_pool(name="ps1", bufs=3, space="PSUM"))
        pst = ctx.enter_context(tc.tile_pool(name="pst", bufs=2, space="PSUM"))
        ps2 = ctx.enter_context(tc.tile_pool(name="ps2", bufs=2, space="PSUM"))
        psd = ctx.enter_context(tc.tile_pool(name="psd", bufs=1, space="PSUM"))

        identb = const.tile([P, P], BF16, tag="identb")
        onesb = const.tile([P, 2], BF16, tag="onesb")
        nc.sync.dma_start(identb[:], id_d[:])
        nc.sync.dma_start(onesb[:], on_d[:])

        for b in range(BPC):
            kb = mpool.tile([P, NT], F32, tag="kb")
            qm = mpool.tile([P, NT], F32, tag="qm")
            nc.sync.dma_start(kb[:], kb_d[b])
            nc.sync.dma_start(qm[:], qm_d[b])

            qt = qpool.tile([P, NT, D], F32, tag="qt")    # raw q -> qn
            ct = cpool.tile([P, NT, D], F32, tag="ct")    # raw context
            cb = cbpool.tile([P, NT, D], BF16, tag="cb")  # bf16 context
            ssqq = mpool.tile([P, NT], F32, tag="ssqq")
            ssqc = mpool.tile([P, NT], F32, tag="ssqc")
            nrmq = mpool.tile([P, NT], F32, tag="nrmq")
            invq = mpool.tile([P, NT], F32, tag="invq")
            sclc = mpool.tile([P, NT], F32, tag="sclc")
            qTb = tpool.tile([P, ND, S], BF16, tag="qTb")
            cTb = tpool.tile([P, ND, S], BF16, tag="cTb")

            # ---- per s-tile: load, norms, qn, casts, transposes ----
            for t in range(NT):
                sl = slice(t * P, (t + 1) * P)
                nc.sync.dma_start(qt[:, t], q_d[b, sl])
                nc.sync.dma_start(ct[:, t], c_d[b, sl])

                scr = spool.tile([P, D], F32, tag="scr")
                nc.vector.tensor_mul(scr[:], qt[:, t], qt[:, t])
                nc.vector.reduce_sum(ssqq[:, t : t + 1], scr[:], axis=AX.X)
                scrc = spool.tile([P, D], F32, tag="scrc")
                nc.scalar.activation(
                    scrc[:], ct[:, t], AF.Square,
                    accum_out=ssqc[:, t : t + 1],
                )
                nc.scalar.activation(
                    nrmq[:, t : t + 1], ssqq[:, t : t + 1], AF.Sqrt
                )
                nc.vector.reciprocal(invq[:, t : t + 1], nrmq[:, t : t + 1])
                nc.vector.tensor_scalar_mul(
                    qt[:, t], qt[:, t], invq[:, t : t + 1]
                )
                nc.sync.dma_start(out_d[b, sl, 0:D], qt[:, t])

                if PHASE < 2:
                    continue
                # bf16 casts feeding the PE (GpSimd is otherwise idle)
                qnb = qbpool.tile([P, D], BF16, tag="qnb")
                nc.gpsimd.tensor_copy(qnb[:], qt[:, t])
                nc.gpsimd.tensor_copy(cb[:, t], ct[:, t])

                ptq = pst.tile([P, ND, P], BF16, tag="pt")
                for dch in range(ND):
                    nc.tensor.transpose(
                        ptq[:, dch], qnb[:, dch * P : (dch + 1) * P],
                        identb[:],
                    )
                nc.scalar.copy(qTb[:, :, sl], ptq[:])
                ptc = pst.tile([P, ND, P], BF16, tag="pt")
                for dch in range(ND):
                    nc.tensor.transpose(
                        ptc[:, dch], cb[:, t, dch * P : (dch + 1) * P],
                        identb[:],
                    )
                nc.vector.tensor_copy(cTb[:, :, sl], ptc[:])

            if PHASE < 3:
                continue
            # context norm scale (folds into sigmoid)
            nc.scalar.activation(sclc[:], ssqc[:], AF.Sqrt)
            nc.vector.reciprocal(sclc[:], sclc[:])

            # ---- mm1: sigT[k, q] = sigmoid(cTb.T @ qTb + keybias) ----
            sg = sgpool.tile([P, NT, S], BF16, tag="sg")
            for kt in range(NT):
                acc0 = ps1.tile([P, 512], F32, tag="acc")
                acc1 = ps1.tile([P, 512], F32, tag="acc")
                acc = [acc0, acc1]
                for dch in range(ND):
                    for qc in range(2):
                        nc.tensor.matmul(
                            acc[qc][:],
                            cTb[:, dch, kt * P : (kt + 1) * P],
                            qTb[:, dch, qc * 512 : (qc + 1) * 512],
                            start=(dch == 0),
                            stop=(dch == ND - 1),
                        )
                for qc in range(2):
                    nc.scalar.activation(
                        sg[:, kt, qc * 512 : (qc + 1) * 512], acc[qc][:],
                        AF.Sigmoid, bias=kb[:, kt : kt + 1],
                        scale=sclc[:, kt : kt + 1],
                    )

            if PHASE < 4:
                continue
            # ---- per q-block: attended + denominator + scores out ----
            for qb in range(NT):
                sl = slice(qb * P, (qb + 1) * P)
                att = ps2.tile([P, 512], F32, tag="att")
                dn = psd.tile([P, 2], F32, tag="dn")
                for kt in range(NT):
                    sgblk = sg[:, kt, sl]
                    nc.tensor.matmul(
                        att[:], sgblk, cb[:, kt],
                        start=(kt == 0), stop=(kt == NT - 1),
                    )
                    nc.tensor.matmul(
                        dn[:], sgblk, onesb[:],
                        start=(kt == 0), stop=(kt == NT - 1),
                    )
                # w = qmask / max(den, 1)
                w = wpool.tile([P, 1], F32, tag="w")
                nc.vector.tensor_scalar_max(w[:], dn[:, 0:1], 1.0)
                nc.vector.reciprocal(w[:], w[:])
                nc.vector.tensor_mul(w[:], w[:], qm[:, qb : qb + 1])

                ao = opool.tile([P, D], F32, tag="ao")
                nc.vector.tensor_scalar_mul(ao[:], att[:], w[:])
                nc.sync.dma_start(out_d[b, sl, D : 2 * D], ao[:])

                so = opool.tile([P, S], F32, tag="so")
                for kg in range(2):
                    ptg = pst.tile([P, 4, P], BF16, tag="pt")
                    for j in range(4):
                        kt = kg * 4 + j
                        nc.tensor.transpose(
                            ptg[:, j], sg[:, kt, sl], identb[:]
                        )
                    if kg == 0:
                        nc.scalar.activation(
                            so[:, 0:512], ptg[:], AF.Copy, scale=w[:]
                        )
                    else:
                        nc.vector.tensor_scalar_mul(
                            so[:, 512:1024], ptg[:], w[:]
                        )
                nc.sync.dma_start(sc_d[b, sl, :], so[:])


_NC_CACHE = {}


def _get_nc():
    if "nc" not in _NC_CACHE:
        _NC_CACHE["nc"] = build_kernel()
    return _NC_CACHE["nc"]


def _host_inputs(context, query, length):
    iot = np.arange(S)
    keymask = iot[None, :] < length[:, None]                      # [B, S]
    kbH = np.where(keymask, np.float32(0.0), NEG).astype(np.float32)
    kbH = np.ascontiguousarray(kbH.reshape(B, NT, P).transpose(0, 2, 1))
    qmH = keymask.astype(np.float32)
    qmH = np.ascontiguousarray(qmH.reshape(B, NT, P).transpose(0, 2, 1))
    identb = np.eye(P, dtype=ml_dtypes.bfloat16)
    onesb = np.ones((P, 2), dtype=ml_dtypes.bfloat16)
    return kbH, qmH, identb, onesb


def kernel(context, query, length):
    context = np.ascontiguousarray(np.asarray(context, dtype=np.float32))
    query = np.ascontiguousarray(np.asarray(query, dtype=np.float32))
    length = np.asarray(length).astype(np.int64)

    kbH, qmH, identb, onesb = _host_inputs(context, query, length)

    in_maps = []
    for c in range(NCORES):
        sl = slice(c * BPC, (c + 1) * BPC)
        in_maps.append(
            {
                "query": np.ascontiguousarray(query[sl]),
                "context": np.ascontiguousarray(context[sl]),
                "keybias": np.ascontiguousarray(kbH[sl]),
                "qmask": np.ascontiguousarray(qmH[sl]),
                "identb": identb,
                "onesb": onesb,
            }
        )

    nc = _get_nc()
    res = run_bass_kernel_spmd(nc, in_maps, list(range(NCORES)))
    _NC_CACHE["last_result"] = res
    out = np.concatenate([res.results[c]["out"] for c in range(NCORES)], axis=0)
    scores = np.concatenate(
        [res.results[c]["scores"] for c in range(NCORES)], axis=0
    )
    return out, scores


# revision 8
# speedup vs baseline: 1.4316x; 1.1833x over previous
"""Trainium2 Bass kernel for BiLinearSigmoidAttention.

Reference math (per batch b, with L = length[b]):
    qn = l2norm(query), cn = l2norm(context)
    raw[q,k] = qn[q] . cn[k]            (masked: k >= L -> -1e30)
    sig = sigmoid(raw)
    den[q] = max(sum_k sig[q,k], 1)
    scores[q,k] = sig[q,k] / den[q]     (rows q >= L zeroed)
    att[q,:] = sum_k scores[q,k] * context[k,:]
    out = concat([qn, att], -1)
returns (out [B,S,2D], scores [B,S,S])

Device mapping (8 NeuronCores, pure data parallel over B=32 -> 4 per core).

Engine plan per batch (PE kept dense; the whole PE path is bf16 since
walrus forbids mixing 32-bit with 16-bit matmul operands; rel-err budget
is 2e-2, bf16 lands ~3e-3):
  - q/context are loaded straight into bf16 via SWDGE casting DMAs (no
    fp32 staging in SBUF, no engine cast passes).
  - context transposes start as soon as each s-tile lands; q is
    normalized in place (qb *= 1/||q||) after a single batched
    sqrt/reciprocal, stored to out via a casting DMA, then transposed.
  - mm1: sigT[k,q] = sigmoid(cTb.T @ qTb + keybias), weights reused
    across the two q-halves (kt->dch->qc loop order); context l2-norm
    folded into the sigmoid per-partition scale; evicts to bf16 sg.
  - mm2: att[q,d] = sgblk.T @ cb; denominator rides the same weights
    via a tiny ones matmul.
  - scores out: PE transposes of bf16 sg blocks, scaled by w=qmask/den
    during PSUM->SBUF eviction (split across ACT and DVE).
"""

import numpy as np
import ml_dtypes

import concourse.bacc as bacc
import concourse.mybir as mybir
import concourse.tile as tile
from concourse.bass_utils import run_bass_kernel_spmd

B, S, D = 32, 1024, 512
NCORES = 8
BPC = B // NCORES          # batches per core
P = 128                    # partitions
NT = S // P                # 8 s-tiles
ND = D // P                # 4 d-chunks
NEG = np.float32(-1e30)

F32 = mybir.dt.float32
F32R = mybir.dt.float32r
BF16 = mybir.dt.bfloat16
AF = mybir.ActivationFunctionType
ALU = mybir.AluOpType
AX = mybir.AxisListType


def build_kernel():
    nc = bacc.Bacc("TRN2", target_bir_lowering=False, debug=False)

    q_d = nc.dram_tensor("query", [BPC, S, D], F32, kind="ExternalInput")
    c_d = nc.dram_tensor("context", [BPC, S, D], F32, kind="ExternalInput")
    # keybias[b, p, kt] = 0 if kt*P+p < L else -1e30
    kb_d = nc.dram_tensor("keybias", [BPC, P, NT], F32, kind="ExternalInput")
    # qmask[b, p, qb] = 1 if qb*P+p < L else 0
    qm_d = nc.dram_tensor("qmask", [BPC, P, NT], F32, kind="ExternalInput")
    id_d = nc.dram_tensor("identb", [P, P], BF16, kind="ExternalInput")
    on_d = nc.dram_tensor("onesb", [P, 2], BF16, kind="ExternalInput")
    out_d = nc.dram_tensor("out", [BPC, S, 2 * D], F32, kind="ExternalOutput")
    sc_d = nc.dram_tensor("scores", [BPC, S, S], F32, kind="ExternalOutput")

    with tile.TileContext(nc) as tc:
        _body(tc, q_d, c_d, kb_d, qm_d, id_d, on_d, out_d, sc_d)
    nc.compile()
    return nc


def _body(tc, q_d, c_d, kb_d, qm_d, id_d, on_d, out_d, sc_d):
    import os

    PHASE = int(os.environ.get("KERNEL_PHASE", "4"))
    nc = tc.nc
    from contextlib import ExitStack

    ctx = ExitStack()
    with ctx:
        const = ctx.enter_context(tc.tile_pool(name="const", bufs=1))
        qpool = ctx.enter_context(tc.tile_pool(name="q", bufs=2))
        cpool = ctx.enter_context(tc.tile_pool(name="c", bufs=2))
        tpool = ctx.enter_context(tc.tile_pool(name="t", bufs=2))
        sgpool = ctx.enter_context(tc.tile_pool(name="sg", bufs=2))
        mpool = ctx.enter_context(tc.tile_pool(name="m", bufs=2))
        spool = ctx.enter_context(tc.tile_pool(name="s", bufs=2))
        opool = ctx.enter_context(tc.tile_pool(name="o", bufs=3))
        wpool = ctx.enter_context(tc.tile_pool(name="w", bufs=4))
        ps1 = ctx.enter_context(tc.tile_pool(name="ps1", bufs=3, space="PSUM"))
        pst = ctx.enter_context(tc.tile_pool(name="pst", bufs=2, space="PSUM"))
        ps2 = ctx.enter_context(tc.tile_pool(name="ps2", bufs=2, space="PSUM"))
        psd = ctx.enter_context(tc.tile_pool(name="psd", bufs=1, space="PSUM"))

        identb = const.tile([P, P], BF16, tag="identb")
        onesb = const.tile([P, 2], BF16, tag="onesb")
        nc.sync.dma_start(identb[:], id_d[:])
        nc.sync.dma_start(onesb[:], on_d[:])

        for b in range(BPC):
            kb = mpool.tile([P, NT], F32, tag="kb")
            qm = mpool.tile([P, NT], F32, tag="qm")
            nc.sync.dma_start(kb[:], kb_d[b])
            nc.sync.dma_start(qm[:], qm_d[b])

            qb = qpool.tile([P, NT, D], BF16, tag="qb")   # bf16 q -> qn
            cb = cpool.tile([P, NT, D], BF16, tag="cb")   # bf16 context
            ssq = mpool.tile([P, 2 * NT], F32, tag="ssq")
            inv = mpool.tile([P, 2 * NT], F32, tag="inv")
            qTb = tpool.tile([P, ND, S], BF16, tag="qTb")
            cTb = tpool.tile([P, ND, S], BF16, tag="cTb")

            # ---- loop A: casting loads, squares, context transposes ----
            for t in range(NT):
                sl = slice(t * P, (t + 1) * P)
                nc.gpsimd.dma_start(qb[:, t], q_d[b, sl])
                nc.gpsimd.dma_start(cb[:, t], c_d[b, sl])

                scr = spool.tile([P, D], BF16, tag="scr")
                nc.vector.tensor_mul(scr[:], qb[:, t], qb[:, t])
                nc.vector.reduce_sum(ssq[:, t : t + 1], scr[:], axis=AX.X)
                scrc = spool.tile([P, D], BF16, tag="scrc")
                nc.vector.tensor_mul(scrc[:], cb[:, t], cb[:, t])
                nc.vector.reduce_sum(
                    ssq[:, NT + t : NT + t + 1], scrc[:], axis=AX.X
                )

                if PHASE < 2:
                    continue
                ptc = pst.tile([P, ND, P], BF16, tag="pt")
                for dch in range(ND):
                    nc.tensor.transpose(
                        ptc[:, dch], cb[:, t, dch * P : (dch + 1) * P],
                        identb[:],
                    )
                nc.scalar.copy(cTb[:, :, sl], ptc[:])

            # batched norms: inv = 1/sqrt(ssq) for both q and c
            nrm = mpool.tile([P, 2 * NT], F32, tag="nrm")
            nc.scalar.activation(nrm[:], ssq[:], AF.Sqrt)
            nc.vector.reciprocal(inv[:], nrm[:])

            # ---- loop B: qn in place, store, q transposes ----
            for t in range(NT):
                sl = slice(t * P, (t + 1) * P)
                nc.vector.tensor_scalar_mul(
                    qb[:, t], qb[:, t], inv[:, t : t + 1]
                )
                nc.gpsimd.dma_start(out_d[b, sl, 0:D], qb[:, t])
                if PHASE < 2:
                    continue
                ptq = pst.tile([P, ND, P], BF16, tag="pt")
                for dch in range(ND):
                    nc.tensor.transpose(
                        ptq[:, dch], qb[:, t, dch * P : (dch + 1) * P],
                        identb[:],
                    )
                nc.scalar.copy(qTb[:, :, sl], ptq[:])

            if PHASE < 3:
                continue
            # ---- mm1: sigT[k, q] = sigmoid(cTb.T @ qTb + keybias) ----
            sg = sgpool.tile([P, NT, S], BF16, tag="sg")
            for kt in range(NT):
                acc0 = ps1.tile([P, 512], F32, tag="acc")
                acc1 = ps1.tile([P, 512], F32, tag="acc")
                acc = [acc0, acc1]
                for dch in range(ND):
                    for qc in range(2):
                        nc.tensor.matmul(
                            acc[qc][:],
                            cTb[:, dch, kt * P : (kt + 1) * P],
                            qTb[:, dch, qc * 512 : (qc + 1) * 512],
                            start=(dch == 0),
                            stop=(dch == ND - 1),
                        )
                for qc in range(2):
                    nc.scalar.activation(
                        sg[:, kt, qc * 512 : (qc + 1) * 512], acc[qc][:],
                        AF.Sigmoid, bias=kb[:, kt : kt + 1],
                        scale=inv[:, NT + kt : NT + kt + 1],
                    )

            if PHASE < 4:
                continue
            # ---- per q-block: attended + denominator + scores out ----
            for qb_i in range(NT):
                sl = slice(qb_i * P, (qb_i + 1) * P)
                att = ps2.tile([P, 512], F32, tag="att")
                dn = psd.tile([P, 2], F32, tag="dn")
                for kt in range(NT):
                    sgblk = sg[:, kt, sl]
                    nc.tensor.matmul(
                        att[:], sgblk, cb[:, kt],
                        start=(kt == 0), stop=(kt == NT - 1),
                    )
                    nc.tensor.matmul(
                        dn[:], sgblk, onesb[:],
                        start=(kt == 0), stop=(kt == NT - 1),
                    )
                # w = qmask / max(den, 1)
                w = wpool.tile([P, 1], F32, tag="w")
                nc.vector.tensor_scalar_max(w[:], dn[:, 0:1], 1.0)
                nc.vector.reciprocal(w[:], w[:])
                nc.vector.tensor_mul(w[:], w[:], qm[:, qb_i : qb_i + 1])

                ao = opool.tile([P, D], F32, tag="ao")
                nc.vector.tensor_scalar_mul(ao[:], att[:], w[:])
                nc.sync.dma_start(out_d[b, sl, D : 2 * D], ao[:])

                so = opool.tile([P, S], F32, tag="so")
                for kg in range(2):
                    ptg = pst.tile([P, 4, P], BF16, tag="pt")
                    for j in range(4):
                        kt = kg * 4 + j
                        nc.tensor.transpose(
                            ptg[:, j], sg[:, kt, sl], identb[:]
                        )
                    if kg == 0:
                        nc.scalar.activation(
                            so[:, 0:512], ptg[:], AF.Copy, scale=w[:]
                        )
                    else:
                        nc.vector.tensor_scalar_mul(
                            so[:, 512:1024], ptg[:], w[:]
                        )
                nc.sync.dma_start(sc_d[b, sl, :], so[:])


_NC_CACHE = {}


def _get_nc():
    if "nc" not in _NC_CACHE:
        _NC_CACHE["nc"] = build_kernel()
    return _NC_CACHE["nc"]


def _host_inputs(context, query, length):
    iot = np.arange(S)
    keymask = iot[None, :] < length[:, None]                      # [B, S]
    kbH = np.where(keymask, np.float32(0.0), NEG).astype(np.float32)
    kbH = np.ascontiguousarray(kbH.reshape(B, NT, P).transpose(0, 2, 1))
    qmH = keymask.astype(np.float32)
    qmH = np.ascontiguousarray(qmH.reshape(B, NT, P).transpose(0, 2, 1))
    identb = np.eye(P, dtype=ml_dtypes.bfloat16)
    onesb = np.ones((P, 2), dtype=ml_dtypes.bfloat16)
    return kbH, qmH, identb, onesb


def kernel(context, query, length):
    context = np.ascontiguousarray(np.asarray(context, dtype=np.float32))
    query = np.ascontiguousarray(np.asarray(query, dtype=np.float32))
    length = np.asarray(length).astype(np.int64)

    kbH, qmH, identb, onesb = _host_inputs(context, query, length)

    in_maps = []
    for c in range(NCORES):
        sl = slice(c * BPC, (c + 1) * BPC)
        in_maps.append(
            {
                "query": np.ascontiguousarray(query[sl]),
                "context": np.ascontiguousarray(context[sl]),
                "keybias": np.ascontiguousarray(kbH[sl]),
                "qmask": np.ascontiguousarray(qmH[sl]),
                "identb": identb,
                "onesb": onesb,
            }
        )

    nc = _get_nc()
    res = run_bass_kernel_spmd(nc, in_maps, list(range(NCORES)))
    _NC_CACHE["last_result"] = res
    out = np.concatenate([res.results[c]["out"] for c in range(NCORES)], axis=0)
    scores = np.concatenate(
        [res.results[c]["scores"] for c in range(NCORES)], axis=0
    )
    return out, scores


# revision 9
# speedup vs baseline: 1.5067x; 1.0525x over previous
"""Trainium2 Bass kernel for BiLinearSigmoidAttention.

Reference math (per batch b, with L = length[b]):
    qn = l2norm(query), cn = l2norm(context)
    raw[q,k] = qn[q] . cn[k]            (masked: k >= L -> -1e30)
    sig = sigmoid(raw)
    den[q] = max(sum_k sig[q,k], 1)
    scores[q,k] = sig[q,k] / den[q]     (rows q >= L zeroed)
    att[q,:] = sum_k scores[q,k] * context[k,:]
    out = concat([qn, att], -1)
returns (out [B,S,2D], scores [B,S,S])

Device mapping (8 NeuronCores, pure data parallel over B=32 -> 4 per core).

Engine plan per batch (PE kept dense; the whole PE path is bf16 since
walrus forbids mixing 32-bit with 16-bit matmul operands; rel-err budget
is 2e-2, bf16 lands ~3e-3):
  - q/context are loaded straight into bf16 via SWDGE casting DMAs (no
    fp32 staging in SBUF, no engine cast passes).
  - context transposes start as soon as each s-tile lands; q is
    normalized in place (qb *= 1/||q||) after a single batched
    sqrt/reciprocal, stored to out via a casting DMA, then transposed.
  - mm1: sigT[k,q] = sigmoid(cTb.T @ qTb + keybias), weights reused
    across the two q-halves (kt->dch->qc loop order); context l2-norm
    folded into the sigmoid per-partition scale; evicts to bf16 sg.
  - mm2: att[q,d] = sgblk.T @ cb; denominator rides the same weights
    via a tiny ones matmul.
  - scores out: PE transposes of bf16 sg blocks, scaled by w=qmask/den
    during PSUM->SBUF eviction (split across ACT and DVE).
"""

import numpy as np
import ml_dtypes

import concourse.bacc as bacc
import concourse.mybir as mybir
import concourse.tile as tile
from concourse.bass_utils import run_bass_kernel_spmd

B, S, D = 32, 1024, 512
NCORES = 8
BPC = B // NCORES          # batches per core
P = 128                    # partitions
NT = S // P                # 8 s-tiles
ND = D // P                # 4 d-chunks
NEG = np.float32(-1e30)

F32 = mybir.dt.float32
F32R = mybir.dt.float32r
BF16 = mybir.dt.bfloat16
FP8 = mybir.dt.float8e4
PM = mybir.MatmulPerfMode
AF = mybir.ActivationFunctionType
ALU = mybir.AluOpType
AX = mybir.AxisListType


def build_kernel():
    nc = bacc.Bacc("TRN2", target_bir_lowering=False, debug=False)

    q_d = nc.dram_tensor("query", [BPC, S, D], F32, kind="ExternalInput")
    c_d = nc.dram_tensor("context", [BPC, S, D], F32, kind="ExternalInput")
    # keybias[b, p, kt] = 0 if kt*P+p < L else -1e30
    kb_d = nc.dram_tensor("keybias", [BPC, P, NT], F32, kind="ExternalInput")
    # qmask[b, p, qb] = 1 if qb*P+p < L else 0
    qm_d = nc.dram_tensor("qmask", [BPC, P, NT], F32, kind="ExternalInput")
    id_d = nc.dram_tensor("identb", [P, P], BF16, kind="ExternalInput")
    on_d = nc.dram_tensor("onesb", [P, 2], BF16, kind="ExternalInput")
    out_d = nc.dram_tensor("out", [BPC, S, 2 * D], F32, kind="ExternalOutput")
    sc_d = nc.dram_tensor("scores", [BPC, S, S], F32, kind="ExternalOutput")

    with tile.TileContext(nc) as tc:
        _body(tc, q_d, c_d, kb_d, qm_d, id_d, on_d, out_d, sc_d)
    nc.compile()
    return nc


def _body(tc, q_d, c_d, kb_d, qm_d, id_d, on_d, out_d, sc_d):
    import os

    PHASE = int(os.environ.get("KERNEL_PHASE", "4"))
    nc = tc.nc
    from contextlib import ExitStack

    ctx = ExitStack()
    with ctx:
        const = ctx.enter_context(tc.tile_pool(name="const", bufs=1))
        qpool = ctx.enter_context(tc.tile_pool(name="q", bufs=2))
        cpool = ctx.enter_context(tc.tile_pool(name="c", bufs=2))
        tpool = ctx.enter_context(tc.tile_pool(name="t", bufs=2))
        sgpool = ctx.enter_context(tc.tile_pool(name="sg", bufs=2))
        mpool = ctx.enter_context(tc.tile_pool(name="m", bufs=2))
        spool = ctx.enter_context(tc.tile_pool(name="s", bufs=2))
        opool = ctx.enter_context(tc.tile_pool(name="o", bufs=3))
        wpool = ctx.enter_context(tc.tile_pool(name="w", bufs=4))
        ps1 = ctx.enter_context(tc.tile_pool(name="ps1", bufs=3, space="PSUM"))
        pst = ctx.enter_context(tc.tile_pool(name="pst", bufs=2, space="PSUM"))
        ps2 = ctx.enter_context(tc.tile_pool(name="ps2", bufs=2, space="PSUM"))
        psd = ctx.enter_context(tc.tile_pool(name="psd", bufs=1, space="PSUM"))

        identb = const.tile([P, P], BF16, tag="identb")
        onesb = const.tile([P, 2], BF16, tag="onesb")
        nc.sync.dma_start(identb[:], id_d[:])
        nc.sync.dma_start(onesb[:], on_d[:])

        for b in range(BPC):
            kb = mpool.tile([P, NT], F32, tag="kb")
            qm = mpool.tile([P, NT], F32, tag="qm")
            nc.sync.dma_start(kb[:], kb_d[b])
            nc.sync.dma_start(qm[:], qm_d[b])

            qb = qpool.tile([P, NT, D], BF16, tag="qb")   # bf16 q -> qn
            cb = cpool.tile([P, NT, D], BF16, tag="cb")   # bf16 context
            ssq = mpool.tile([P, 2 * NT], F32, tag="ssq")
            inv = mpool.tile([P, 2 * NT], F32, tag="inv")
            qT8 = tpool.tile([P, ND, S], FP8, tag="qT8")
            cT8 = tpool.tile([P, ND, S], FP8, tag="cT8")

            # ---- loop A: casting loads, squares, context transposes ----
            for t in range(NT):
                sl = slice(t * P, (t + 1) * P)
                nc.gpsimd.dma_start(qb[:, t], q_d[b, sl])
                nc.gpsimd.dma_start(cb[:, t], c_d[b, sl])

                scr = spool.tile([P, D], BF16, tag="scr")
                nc.vector.tensor_mul(scr[:], qb[:, t], qb[:, t])
                nc.vector.reduce_sum(ssq[:, t : t + 1], scr[:], axis=AX.X)
                scrc = spool.tile([P, D], BF16, tag="scrc")
                nc.vector.tensor_mul(scrc[:], cb[:, t], cb[:, t])
                nc.vector.reduce_sum(
                    ssq[:, NT + t : NT + t + 1], scrc[:], axis=AX.X
                )

                if PHASE < 2:
                    continue
                ptc = pst.tile([P, ND, P], BF16, tag="pt")
                for dch in range(ND):
                    nc.tensor.transpose(
                        ptc[:, dch], cb[:, t, dch * P : (dch + 1) * P],
                        identb[:],
                    )
                nc.scalar.activation(
                    cT8[:, :, sl], ptc[:], AF.Copy, scale=8.0
                )

            # batched norms: inv = 1/sqrt(ssq) for both q and c
            nrm = mpool.tile([P, 2 * NT], F32, tag="nrm")
            nc.scalar.activation(nrm[:], ssq[:], AF.Sqrt)
            nc.vector.reciprocal(inv[:], nrm[:])
            # mm1 runs on fp8 inputs prescaled by 8 (q and c) -> /64 here
            nc.vector.tensor_scalar_mul(
                inv[:, NT : 2 * NT], inv[:, NT : 2 * NT], 1.0 / 64.0
            )

            # ---- loop B: qn in place, store, q transposes ----
            for t in range(NT):
                sl = slice(t * P, (t + 1) * P)
                nc.vector.tensor_scalar_mul(
                    qb[:, t], qb[:, t], inv[:, t : t + 1]
                )
                nc.gpsimd.dma_start(out_d[b, sl, 0:D], qb[:, t])
                if PHASE < 2:
                    continue
                ptq = pst.tile([P, ND, P], BF16, tag="pt")
                for dch in range(ND):
                    nc.tensor.transpose(
                        ptq[:, dch], qb[:, t, dch * P : (dch + 1) * P],
                        identb[:],
                    )
                nc.scalar.activation(
                    qT8[:, :, sl], ptq[:], AF.Copy, scale=8.0
                )

            if PHASE < 3:
                continue
            # ---- mm1: sigT[k, q] = sigmoid(cTb.T @ qTb + keybias) ----
            sg = sgpool.tile([P, NT, S], BF16, tag="sg")
            for kt in range(NT):
                acc0 = ps1.tile([P, 512], F32, tag="acc")
                acc1 = ps1.tile([P, 512], F32, tag="acc")
                acc = [acc0, acc1]
                for dg in range(ND // 2):
                    for qc in range(2):
                        nc.tensor.matmul(
                            acc[qc][:],
                            cT8[:, 2 * dg : 2 * dg + 2, kt * P : (kt + 1) * P],
                            qT8[:, 2 * dg : 2 * dg + 2, qc * 512 : (qc + 1) * 512],
                            start=(dg == 0),
                            stop=(dg == ND // 2 - 1),
                            perf_mode=PM.DoubleRow,
                        )
                for qc in range(2):
                    nc.scalar.activation(
                        sg[:, kt, qc * 512 : (qc + 1) * 512], acc[qc][:],
                        AF.Sigmoid, bias=kb[:, kt : kt + 1],
                        scale=inv[:, NT + kt : NT + kt + 1],
                    )

            if PHASE < 4:
                continue
            # ---- per q-block: attended + denominator + scores out ----
            for qb_i in range(NT):
                sl = slice(qb_i * P, (qb_i + 1) * P)
                att = ps2.tile([P, 512], F32, tag="att")
                dn = psd.tile([P, 2], F32, tag="dn")
                for kt in range(NT):
                    sgblk = sg[:, kt, sl]
                    nc.tensor.matmul(
                        att[:], sgblk, cb[:, kt],
                        start=(kt == 0), stop=(kt == NT - 1),
                    )
                    nc.tensor.matmul(
                        dn[:], sgblk, onesb[:],
                        start=(kt == 0), stop=(kt == NT - 1),
                    )
                # w = qmask / max(den, 1)
                w = wpool.tile([P, 1], F32, tag="w")
                nc.vector.tensor_scalar_max(w[:], dn[:, 0:1], 1.0)
                nc.vector.reciprocal(w[:], w[:])
                nc.vector.tensor_mul(w[:], w[:], qm[:, qb_i : qb_i + 1])

                ao = opool.tile([P, D], F32, tag="ao")
                nc.vector.tensor_scalar_mul(ao[:], att[:], w[:])
                nc.sync.dma_start(out_d[b, sl, D : 2 * D], ao[:])

                so = opool.tile([P, S], F32, tag="so")
                for kg in range(2):
                    ptg = pst.tile([P, 4, P], BF16, tag="pt")
                    for j in range(4):
                        kt = kg * 4 + j
                        nc.tensor.transpose(
                            ptg[:, j], sg[:, kt, sl], identb[:]
                        )
                    if kg == 0:
                        nc.scalar.activation(
                            so[:, 0:512], ptg[:], AF.Copy, scale=w[:]
                        )
                    else:
                        nc.vector.tensor_scalar_mul(
                            so[:, 512:1024], ptg[:], w[:]
                        )
                nc.sync.dma_start(sc_d[b, sl, :], so[:])


_NC_CACHE = {}


def _get_nc():
    if "nc" not in _NC_CACHE:
        _NC_CACHE["nc"] = build_kernel()
    return _NC_CACHE["nc"]


def _host_inputs(context, query, length):
    iot = np.arange(S)
    keymask = iot[None, :] < length[:, None]                      # [B, S]
    kbH = np.where(keymask, np.float32(0.0), NEG).astype(np.float32)
    kbH = np.ascontiguousarray(kbH.reshape(B, NT, P).transpose(0, 2, 1))
    qmH = keymask.astype(np.float32)
    qmH = np.ascontiguousarray(qmH.reshape(B, NT, P).transpose(0, 2, 1))
    identb = np.eye(P, dtype=ml_dtypes.bfloat16)
    onesb = np.ones((P, 2), dtype=ml_dtypes.bfloat16)
    return kbH, qmH, identb, onesb


def kernel(context, query, length):
    context = np.ascontiguousarray(np.asarray(context, dtype=np.float32))
    query = np.ascontiguousarray(np.asarray(query, dtype=np.float32))
    length = np.asarray(length).astype(np.int64)

    kbH, qmH, identb, onesb = _host_inputs(context, query, length)

    in_maps = []
    for c in range(NCORES):
        sl = slice(c * BPC, (c + 1) * BPC)
        in_maps.append(
            {
                "query": np.ascontiguousarray(query[sl]),
                "context": np.ascontiguousarray(context[sl]),
                "keybias": np.ascontiguousarray(kbH[sl]),
                "qmask": np.ascontiguousarray(qmH[sl]),
                "identb": identb,
                "onesb": onesb,
            }
        )

    nc = _get_nc()
    res = run_bass_kernel_spmd(nc, in_maps, list(range(NCORES)))
    _NC_CACHE["last_result"] = res
    out = np.concatenate([res.results[c]["out"] for c in range(NCORES)], axis=0)
    scores = np.concatenate(
        [res.results[c]["scores"] for c in range(NCORES)], axis=0
    )
    return out, scores


# revision 11
# speedup vs baseline: 1.6132x; 1.0707x over previous
"""Trainium2 Bass kernel for BiLinearSigmoidAttention.

Reference math (per batch b, with L = length[b]):
    qn = l2norm(query), cn = l2norm(context)
    raw[q,k] = qn[q] . cn[k]            (masked: k >= L -> -1e30)
    sig = sigmoid(raw)
    den[q] = max(sum_k sig[q,k], 1)
    scores[q,k] = sig[q,k] / den[q]     (rows q >= L zeroed)
    att[q,:] = sum_k scores[q,k] * context[k,:]
    out = concat([qn, att], -1)
returns (out [B,S,2D], scores [B,S,S])

Device mapping (8 NeuronCores, pure data parallel over B=32 -> 4 per core).

Engine plan per batch (PE kept dense; the whole PE path is bf16 since
walrus forbids mixing 32-bit with 16-bit matmul operands; rel-err budget
is 2e-2, bf16 lands ~3e-3):
  - q/context are loaded straight into bf16 via SWDGE casting DMAs (no
    fp32 staging in SBUF, no engine cast passes).
  - context transposes start as soon as each s-tile lands; q is
    normalized in place (qb *= 1/||q||) after a single batched
    sqrt/reciprocal, stored to out via a casting DMA, then transposed.
  - mm1: sigT[k,q] = sigmoid(cTb.T @ qTb + keybias), weights reused
    across the two q-halves (kt->dch->qc loop order); context l2-norm
    folded into the sigmoid per-partition scale; evicts to bf16 sg.
  - mm2: att[q,d] = sgblk.T @ cb; denominator rides the same weights
    via a tiny ones matmul.
  - scores out: PE transposes of bf16 sg blocks, scaled by w=qmask/den
    during PSUM->SBUF eviction (split across ACT and DVE).
"""

import numpy as np
import ml_dtypes

import concourse.bacc as bacc
import concourse.mybir as mybir
import concourse.tile as tile
from concourse.bass_utils import run_bass_kernel_spmd

B, S, D = 32, 1024, 512
NCORES = 8
BPC = B // NCORES          # batches per core
P = 128                    # partitions
NT = S // P                # 8 s-tiles
ND = D // P                # 4 d-chunks
NEG = np.float32(-1e30)

F32 = mybir.dt.float32
F32R = mybir.dt.float32r
BF16 = mybir.dt.bfloat16
FP8 = mybir.dt.float8e4
PM = mybir.MatmulPerfMode
AF = mybir.ActivationFunctionType
ALU = mybir.AluOpType
AX = mybir.AxisListType


def build_kernel():
    nc = bacc.Bacc("TRN2", target_bir_lowering=False, debug=False)

    q_d = nc.dram_tensor("query", [BPC, S, D], F32, kind="ExternalInput")
    c_d = nc.dram_tensor("context", [BPC, S, D], F32, kind="ExternalInput")
    # keybias[b, p, kt] = 0 if kt*P+p < L else -1e30
    kb_d = nc.dram_tensor("keybias", [BPC, P, NT], F32, kind="ExternalInput")
    # qmask[b, p, qb] = 1 if qb*P+p < L else 0
    qm_d = nc.dram_tensor("qmask", [BPC, P, NT], F32, kind="ExternalInput")
    id_d = nc.dram_tensor("identb", [P, P], BF16, kind="ExternalInput")
    on_d = nc.dram_tensor("onesb", [P, 2], BF16, kind="ExternalInput")
    out_d = nc.dram_tensor("out", [BPC, S, 2 * D], F32, kind="ExternalOutput")
    sc_d = nc.dram_tensor("scores", [BPC, S, S], F32, kind="ExternalOutput")

    with tile.TileContext(nc) as tc:
        _body(tc, q_d, c_d, kb_d, qm_d, id_d, on_d, out_d, sc_d)
    nc.compile()
    return nc


def _body(tc, q_d, c_d, kb_d, qm_d, id_d, on_d, out_d, sc_d):
    nc = tc.nc
    from contextlib import ExitStack

    ctx = ExitStack()
    with ctx:
        const = ctx.enter_context(tc.tile_pool(name="const", bufs=1))
        qpool = ctx.enter_context(tc.tile_pool(name="q", bufs=2))
        cpool = ctx.enter_context(tc.tile_pool(name="c", bufs=2))
        tpool = ctx.enter_context(tc.tile_pool(name="t", bufs=2))
        sgpool = ctx.enter_context(tc.tile_pool(name="sg", bufs=2))
        mpool = ctx.enter_context(tc.tile_pool(name="m", bufs=2))
        spool = ctx.enter_context(tc.tile_pool(name="s", bufs=2))
        opool = ctx.enter_context(tc.tile_pool(name="o", bufs=3))
        wpool = ctx.enter_context(tc.tile_pool(name="w", bufs=4))
        ps1 = ctx.enter_context(tc.tile_pool(name="ps1", bufs=2, space="PSUM"))
        pst = ctx.enter_context(tc.tile_pool(name="pst", bufs=2, space="PSUM"))
        ps2 = ctx.enter_context(tc.tile_pool(name="ps2", bufs=2, space="PSUM"))
        psd = ctx.enter_context(tc.tile_pool(name="psd", bufs=2, space="PSUM"))

        identb = const.tile([P, P], BF16, tag="identb")
        onesb = const.tile([P, 2], BF16, tag="onesb")
        nc.sync.dma_start(identb[:], id_d[:])
        nc.sync.dma_start(onesb[:], on_d[:])

        st = {}  # per-batch live tiles

        def emit_loads(b):
            kb = mpool.tile([P, NT], F32, tag="kb")
            qm = mpool.tile([P, NT], F32, tag="qm")
            nc.sync.dma_start(kb[:], kb_d[b])
            nc.sync.dma_start(qm[:], qm_d[b])
            qb = qpool.tile([P, NT, D], BF16, tag="qb")
            cb = cpool.tile([P, NT, D], BF16, tag="cb")
            for t in range(NT):
                sl = slice(t * P, (t + 1) * P)
                nc.gpsimd.dma_start(qb[:, t], q_d[b, sl])
                nc.gpsimd.dma_start(cb[:, t], c_d[b, sl])
            st[b] = {"kb": kb, "qm": qm, "qb": qb, "cb": cb}

        def emit_loop_a(b):
            # squares (DVE) + context transposes -> fp8 cT8 (x8 prescale)
            s = st[b]
            ssq = mpool.tile([P, 2 * NT], F32, tag="ssq")
            cT8 = tpool.tile([P, ND, S], FP8, tag="cT8")
            s["ssq"], s["cT8"] = ssq, cT8
            for t in range(NT):
                sl = slice(t * P, (t + 1) * P)
                scr = spool.tile([P, D], BF16, tag="scr")
                nc.vector.tensor_mul(scr[:], s["qb"][:, t], s["qb"][:, t])
                nc.vector.reduce_sum(ssq[:, t : t + 1], scr[:], axis=AX.X)
                scrc = spool.tile([P, D], BF16, tag="scrc")
                nc.vector.tensor_mul(scrc[:], s["cb"][:, t], s["cb"][:, t])
                nc.vector.reduce_sum(
                    ssq[:, NT + t : NT + t + 1], scrc[:], axis=AX.X
                )
                ptc = pst.tile([P, ND, P], BF16, tag="pt")
                for dch in range(ND):
                    nc.tensor.transpose(
                        ptc[:, dch], s["cb"][:, t, dch * P : (dch + 1) * P],
                        identb[:],
                    )
                nc.scalar.activation(
                    cT8[:, :, sl], ptc[:], AF.Copy, scale=8.0
                )

        def emit_norms(b):
            s = st[b]
            nrm = mpool.tile([P, 2 * NT], F32, tag="nrm")
            inv = mpool.tile([P, 2 * NT], F32, tag="inv")
            nc.scalar.activation(nrm[:], s["ssq"][:], AF.Sqrt)
            nc.vector.reciprocal(inv[:], nrm[:])
            # mm1 runs on fp8 inputs prescaled by 8 (q and c) -> /64 here
            nc.vector.tensor_scalar_mul(
                inv[:, NT : 2 * NT], inv[:, NT : 2 * NT], 1.0 / 64.0
            )
            s["inv"] = inv

        def emit_loop_b(b):
            # qn in place (bf16), casting store, q transposes -> fp8 qT8
            s = st[b]
            qT8 = tpool.tile([P, ND, S], FP8, tag="qT8")
            s["qT8"] = qT8
            inv = s["inv"]
            for t in range(NT):
                sl = slice(t * P, (t + 1) * P)
                nc.vector.tensor_scalar_mul(
                    s["qb"][:, t], s["qb"][:, t], inv[:, t : t + 1]
                )
                nc.gpsimd.dma_start(out_d[b, sl, 0:D], s["qb"][:, t])
                ptq = pst.tile([P, ND, P], BF16, tag="pt")
                for dch in range(ND):
                    nc.tensor.transpose(
                        ptq[:, dch], s["qb"][:, t, dch * P : (dch + 1) * P],
                        identb[:],
                    )
                nc.scalar.activation(
                    qT8[:, :, sl], ptq[:], AF.Copy, scale=8.0
                )

        def emit_mm1_slot(b, kt):
            # sigT[k, q-halves] for one kt: fp8 DoubleRow, sigmoid evict
            s = st[b]
            acc0 = ps1.tile([P, 512], F32, tag="acc")
            acc1 = ps1.tile([P, 512], F32, tag="acc")
            acc = [acc0, acc1]
            for dg in range(ND // 2):
                for qc in range(2):
                    nc.tensor.matmul(
                        acc[qc][:],
                        s["cT8"][:, 2 * dg : 2 * dg + 2, kt * P : (kt + 1) * P],
                        s["qT8"][:, 2 * dg : 2 * dg + 2, qc * 512 : (qc + 1) * 512],
                        start=(dg == 0),
                        stop=(dg == ND // 2 - 1),
                        perf_mode=PM.DoubleRow,
                    )
            for qc in range(2):
                nc.scalar.activation(
                    s["sg"][:, kt, qc * 512 : (qc + 1) * 512], acc[qc][:],
                    AF.Sigmoid, bias=s["kb"][:, kt : kt + 1],
                    scale=s["inv"][:, NT + kt : NT + kt + 1],
                )

        def emit_mm2_slot(b, qb_i):
            # attended + denominator + scores out for one q block
            s = st[b]
            sg, cb, qm = s["sg"], s["cb"], s["qm"]
            sl = slice(qb_i * P, (qb_i + 1) * P)
            att = ps2.tile([P, 512], F32, tag="att")
            dn = psd.tile([P, 2], F32, tag="dn")
            for kt in range(NT):
                sgblk = sg[:, kt, sl]
                nc.tensor.matmul(
                    att[:], sgblk, cb[:, kt],
                    start=(kt == 0), stop=(kt == NT - 1),
                )
                nc.tensor.matmul(
                    dn[:], sgblk, onesb[:],
                    start=(kt == 0), stop=(kt == NT - 1),
                )
            w = wpool.tile([P, 1], F32, tag="w")
            nc.vector.tensor_scalar_max(w[:], dn[:, 0:1], 1.0)
            nc.vector.reciprocal(w[:], w[:])
            nc.vector.tensor_mul(w[:], w[:], qm[:, qb_i : qb_i + 1])

            ao = opool.tile([P, D], F32, tag="ao")
            nc.vector.tensor_scalar_mul(ao[:], att[:], w[:])
            nc.sync.dma_start(out_d[b, sl, D : 2 * D], ao[:])

            so = opool.tile([P, S], F32, tag="so")
            for kg in range(2):
                ptg = pst.tile([P, 4, P], BF16, tag="pt")
                for j in range(4):
                    kt = kg * 4 + j
                    nc.tensor.transpose(ptg[:, j], sg[:, kt, sl], identb[:])
                if kg == 0:
                    nc.scalar.activation(
                        so[:, 0:512], ptg[:], AF.Copy, scale=w[:]
                    )
                else:
                    nc.vector.tensor_scalar_mul(
                        so[:, 512:1024], ptg[:], w[:]
                    )
            nc.sync.dma_start(sc_d[b, sl, :], so[:])

        # ---- pipelined schedule ----
        emit_loads(0)
        emit_loop_a(0)
        for b in range(BPC):
            if b + 1 < BPC:
                emit_loads(b + 1)
            emit_norms(b)
            emit_loop_b(b)
            sg_tile = sgpool.tile([P, NT, S], BF16, tag="sg")
            st[b]["sg"] = sg_tile
            # interleave: mm1 of b with mm2+scores of b-1 (keeps PE fed
            # while ACT drains the sigmoid evictions)
            for i in range(NT):
                emit_mm1_slot(b, i)
                if b - 1 >= 0:
                    emit_mm2_slot(b - 1, i)
            if b - 1 >= 0:
                del st[b - 1]
            if b + 1 < BPC:
                emit_loop_a(b + 1)
        for i in range(NT):
            emit_mm2_slot(BPC - 1, i)


_NC_CACHE = {}


def _get_nc():
    if "nc" not in _NC_CACHE:
        _NC_CACHE["nc"] = build_kernel()
    return _NC_CACHE["nc"]


def _host_inputs(context, query, length):
    iot = np.arange(S)
    keymask = iot[None, :] < length[:, None]                      # [B, S]
    kbH = np.where(keymask, np.float32(0.0), NEG).astype(np.float32)
    kbH = np.ascontiguousarray(kbH.reshape(B, NT, P).transpose(0, 2, 1))
    qmH = keymask.astype(np.float32)
    qmH = np.ascontiguousarray(qmH.reshape(B, NT, P).transpose(0, 2, 1))
    identb = np.eye(P, dtype=ml_dtypes.bfloat16)
    onesb = np.ones((P, 2), dtype=ml_dtypes.bfloat16)
    return kbH, qmH, identb, onesb


def kernel(context, query, length):
    context = np.ascontiguousarray(np.asarray(context, dtype=np.float32))
    query = np.ascontiguousarray(np.asarray(query, dtype=np.float32))
    length = np.asarray(length).astype(np.int64)

    kbH, qmH, identb, onesb = _host_inputs(context, query, length)

    in_maps = []
    for c in range(NCORES):
        sl = slice(c * BPC, (c + 1) * BPC)
        in_maps.append(
            {
                "query": np.ascontiguousarray(query[sl]),
                "context": np.ascontiguousarray(context[sl]),
                "keybias": np.ascontiguousarray(kbH[sl]),
                "qmask": np.ascontiguousarray(qmH[sl]),
                "identb": identb,
                "onesb": onesb,
            }
        )

    nc = _get_nc()
    res = run_bass_kernel_spmd(nc, in_maps, list(range(NCORES)))
    _NC_CACHE["last_result"] = res
    out = np.concatenate([res.results[c]["out"] for c in range(NCORES)], axis=0)
    scores = np.concatenate(
        [res.results[c]["scores"] for c in range(NCORES)], axis=0
    )
    return out, scores
